# revision 4
# baseline (speedup 1.0000x reference)
"""VariableSelectionNetwork Trainium2 kernel (8-core data parallel).

Reference computation per row n (F=32 features, H=64 hidden):
    t[n,f,h] = feat[n,f]*W_feat[f,h] + b_feat[f,h]
    gates    = softmax(t.flat @ W_gate + b_gate)        # over f
    out[n,h] = sum_f t[n,f,h] * gates[n,f]

Algebraic collapse (exact, just reassociated):
    logits = feat @ A + c        A[f,g] = sum_h W_feat[f,h]*W_gate[f*H+h,g]
                                 c      = b_feat.flat @ W_gate + b_gate
    e      = exp(logits)         (logits are O(1); no max-shift needed)
    gates  = e * (1 / sum_f e)
    out    = (feat*gates) @ W_feat + gates @ b_feat

So the device streams feat [N,32] in and out [N,64] back — memory-bound.
"""

import sys

sys.path.insert(0, "/opt/trn_rl_repo")

import numpy as np

from concourse import bacc, masks, mybir, tile
from concourse.bass_utils import run_bass_kernel_spmd

B, S, F, H = 32, 512, 32, 64
N = B * S
NCORES = 8
NC_ROWS = N // NCORES  # 2048 rows per core
P = 128
SLAB = 512             # rows per slab iteration
RPP = SLAB // P        # rows per partition per slab (4)
NSLAB = NC_ROWS // SLAB
F32 = mybir.dt.float32
EXP = mybir.ActivationFunctionType.Exp

_NC_CACHE = {}


def _build_nc():
    nc = bacc.Bacc("TRN2", target_bir_lowering=False, debug=False, num_devices=NCORES)

    feat_d = nc.dram_tensor("feat", [NC_ROWS, F], F32, kind="ExternalInput").ap()
    a_d = nc.dram_tensor("Amat", [F, F], F32, kind="ExternalInput").ap()
    c_d = nc.dram_tensor("cvec", [F, 1], F32, kind="ExternalInput").ap()
    wf_d = nc.dram_tensor("Wf", [F, H], F32, kind="ExternalInput").ap()
    bf_d = nc.dram_tensor("Bf", [F, H], F32, kind="ExternalInput").ap()
    out_d = nc.dram_tensor("out", [NC_ROWS, H], F32, kind="ExternalOutput").ap()

    with tile.TileContext(nc) as tc:
        with (
            tc.tile_pool(name="const", bufs=1) as cpool,
            tc.tile_pool(name="io", bufs=2) as io,
            tc.tile_pool(name="work", bufs=2) as work,
            tc.tile_pool(name="ps", bufs=2, space="PSUM") as ps,
            tc.tile_pool(name="ps1", bufs=1, space="PSUM") as ps1,
        ):
            ident = cpool.tile([P, P], F32)
            masks.make_identity(nc, ident[:])
            a_t = cpool.tile([F, F], F32)
            nc.sync.dma_start(a_t[:], a_d)
            c_t = cpool.tile([F, 1], F32)
            nc.sync.dma_start(c_t[:], c_d)
            wf_t = cpool.tile([F, H], F32)
            nc.sync.dma_start(wf_t[:], wf_d)
            bf_t = cpool.tile([F, H], F32)
            nc.sync.dma_start(bf_t[:], bf_d)
            ones_col = cpool.tile([F, 1], F32)
            nc.any.memset(ones_col[:], 1.0)
            ones_row = cpool.tile([1, F], F32)
            nc.any.memset(ones_row[:], 1.0)

            # row index = s*SLAB + p*RPP + r  (contiguous per partition)
            feat_r = feat_d.rearrange("(s p r) f -> s p (r f)", p=P, r=RPP)
            out_r = out_d.rearrange("(s p r) h -> s p (r h)", p=P, r=RPP)

            for s in range(NSLAB):
                ftile = io.tile([P, RPP * F], F32, tag="fin")
                nc.sync.dma_start(ftile[:], feat_r[s])

                featT = work.tile([F, SLAB], F32, tag="featT")
                for r in range(RPP):
                    tp = ps.tile([F, P], F32, tag="tp")
                    nc.tensor.transpose(tp[:], ftile[:, r * F : (r + 1) * F], ident[:])
                    nc.scalar.copy(featT[:, r * P : (r + 1) * P], tp[:])

                lg = ps.tile([F, SLAB], F32, tag="lg")
                nc.tensor.matmul(lg[:], a_t[:], featT[:])

                et = work.tile([F, SLAB], F32, tag="et")
                nc.scalar.activation(et[:], lg[:], EXP, bias=c_t[:])

                dsum = ps1.tile([1, SLAB], F32, tag="dsum")
                nc.tensor.matmul(dsum[:], ones_col[:], et[:])
                rcp = work.tile([1, SLAB], F32, tag="rcp")
                nc.vector.reciprocal(rcp[:], dsum[:])
                rb = ps1.tile([F, SLAB], F32, tag="rb")
                nc.tensor.matmul(rb[:], ones_row[:], rcp[:])

                gatesT = work.tile([F, SLAB], F32, tag="gatesT")
                nc.vector.tensor_mul(gatesT[:], et[:], rb[:])
                wgfT = work.tile([F, SLAB], F32, tag="wgfT")
                nc.vector.tensor_mul(wgfT[:], featT[:], gatesT[:])

                otile = io.tile([P, RPP * H], F32, tag="oout")
                for r in range(RPP):
                    op = ps.tile([P, H], F32, tag="op")
                    nc.tensor.matmul(
                        op[:], wgfT[:, r * P : (r + 1) * P], wf_t[:],
                        start=True, stop=False,
                    )
                    nc.tensor.matmul(
                        op[:], gatesT[:, r * P : (r + 1) * P], bf_t[:],
                        start=False, stop=True,
                    )
                    nc.any.tensor_copy(otile[:, r * H : (r + 1) * H], op[:])
                nc.sync.dma_start(out_r[s], otile[:])

    nc.compile()
    return nc


def _get_nc():
    if "nc" not in _NC_CACHE:
        _NC_CACHE["nc"] = _build_nc()
    return _NC_CACHE["nc"]


def _prep_params(W_feat, b_feat, W_gate, b_gate):
    wf = np.asarray(W_feat, np.float64)
    wg = np.asarray(W_gate, np.float64).reshape(F, H, F)
    a = np.einsum("fh,fhg->fg", wf, wg).astype(np.float32)
    c = (
        np.asarray(b_feat, np.float64).reshape(-1) @ np.asarray(W_gate, np.float64)
        + np.asarray(b_gate, np.float64)
    ).astype(np.float32).reshape(F, 1)
    return a, c


def kernel(features, W_feat, b_feat, W_gate, b_gate):
    a, c = _prep_params(W_feat, b_feat, W_gate, b_gate)
    featf = np.ascontiguousarray(np.asarray(features, np.float32).reshape(N, F))
    wf = np.ascontiguousarray(np.asarray(W_feat, np.float32))
    bf = np.ascontiguousarray(np.asarray(b_feat, np.float32))
    nc = _get_nc()
    in_maps = [
        {
            "feat": featf[i * NC_ROWS : (i + 1) * NC_ROWS],
            "Amat": a,
            "cvec": c,
            "Wf": wf,
            "Bf": bf,
        }
        for i in range(NCORES)
    ]
    res = run_bass_kernel_spmd(nc, in_maps, list(range(NCORES))).results
    out = np.concatenate([res[i]["out"] for i in range(NCORES)], axis=0)
    return out.reshape(B, S, H)


# revision 10
# speedup vs baseline: 1.6396x; 1.6396x over previous
"""VariableSelectionNetwork Trainium2 kernel (8-core data parallel).

Reference computation per row n (F=32 features, H=64 hidden):
    t[n,f,h] = feat[n,f]*W_feat[f,h] + b_feat[f,h]
    gates    = softmax(t.flat @ W_gate + b_gate)        # over f
    out[n,h] = sum_f t[n,f,h] * gates[n,f]

Algebraic collapse (exact, just reassociated):
    logits = feat @ A + c        A[f,g] = sum_h W_feat[f,h]*W_gate[f*H+h,g]
                                 c      = b_feat.flat @ W_gate + b_gate
    e      = exp(logits)         (logits are O(1); no max-shift needed)
    gates  = e * (1 / sum_f e)
    out    = (feat*gates) @ W_feat + gates @ b_feat

Device dataflow per core (2048 rows), "blocked transpose" scheme:
    ftile[p, (rb f)] <- DMA          (row = p*16+rb; 2KB/partition contiguous)
    featT = StreamTranspose(ftile)   -> featT[(pb f), (rb pl)], row=(pb*32+pl)*16+rb
    lg    = blockdiag(A,x4).T @ featT            (one matmul, all 2048 rows)
    et    = exp(lg + c_rep)                      (ACT, bias per partition)
    dsum  = blockdiag(ones32,x4).T @ et          -> [4, 512] row sums over f
    rcp   = 1/dsum                               (DVE)
    rcp_b = sel.T @ rcp                          -> broadcast [128, 512]
    gatesT= et * rcp_b ; wgfT = featT * gatesT   (DVE)
    outT_pb = W_feat_rep.T @ wgfT_pb + b_feat_rep.T @ gatesT_pb   (per 32-block)
    otile[p, (rb h)] = StreamTranspose(outT)     -> DMA out (4KB/partition)
"""

import sys

sys.path.insert(0, "/opt/trn_rl_repo")

import numpy as np

from concourse import bacc, mybir, tile
from concourse.bass_utils import run_bass_kernel_spmd

B, S, F, H = 32, 512, 32, 64
N = B * S
NCORES = 8
NC_ROWS = N // NCORES  # 2048 rows per core
P = 128
NPB = P // F           # 4 partition blocks
RPP = NC_ROWS // P     # 16 rows per partition
F32 = mybir.dt.float32
F32R = mybir.dt.float32r
EXP = mybir.ActivationFunctionType.Exp

USE_F32R = False

_NC_CACHE = {}


def _mm_dt(ap):
    return ap.bitcast(F32R) if USE_F32R else ap


def _build_nc():
    nc = bacc.Bacc("TRN2", target_bir_lowering=False, debug=False, num_devices=NCORES)

    feat_d = nc.dram_tensor("feat", [NC_ROWS, F], F32, kind="ExternalInput").ap()
    bda_d = nc.dram_tensor("bdA", [P, P], F32, kind="ExternalInput").ap()
    crep_d = nc.dram_tensor("crep", [P, 1], F32, kind="ExternalInput").ap()
    bd1_d = nc.dram_tensor("bd1", [P, NPB], F32, kind="ExternalInput").ap()
    sel_d = nc.dram_tensor("sel", [NPB, P], F32, kind="ExternalInput").ap()
    bdw_d = [
        nc.dram_tensor(f"bdw{hb}", [P, P], F32, kind="ExternalInput").ap()
        for hb in range(2)
    ]
    bdb_d = [
        nc.dram_tensor(f"bdb{hb}", [P, P], F32, kind="ExternalInput").ap()
        for hb in range(2)
    ]
    out_d = nc.dram_tensor("out", [NC_ROWS, H], F32, kind="ExternalOutput").ap()

    NW = NC_ROWS // NPB  # 512 columns in transposed domain

    with tile.TileContext(nc) as tc:
        with (
            tc.tile_pool(name="const", bufs=1) as cpool,
            tc.tile_pool(name="work", bufs=1) as work,
            tc.tile_pool(name="ps", bufs=1, space="PSUM") as ps,
        ):
            bda_t = cpool.tile([P, P], F32)
            nc.sync.dma_start(bda_t[:], bda_d)
            crep_t = cpool.tile([P, 1], F32)
            nc.sync.dma_start(crep_t[:], crep_d)
            bd1_t = cpool.tile([P, NPB], F32)
            nc.sync.dma_start(bd1_t[:], bd1_d)
            sel_t = cpool.tile([NPB, P], F32)
            nc.sync.dma_start(sel_t[:], sel_d)
            bdw_t = []
            bdb_t = []
            for hb in range(2):
                w = cpool.tile([P, P], F32, tag=f"bdw{hb}")
                nc.sync.dma_start(w[:], bdw_d[hb])
                bdw_t.append(w)
                b = cpool.tile([P, P], F32, tag=f"bdb{hb}")
                nc.sync.dma_start(b[:], bdb_d[hb])
                bdb_t.append(b)

            feat_r = feat_d.rearrange("(p r) f -> p (r f)", p=P)   # [128, 512]
            out_r = out_d.rearrange("(p r) h -> p (r h)", p=P)     # [128, 1024]

            ftile = work.tile([P, RPP * F], F32, tag="fin")
            nc.sync.dma_start(ftile[:], feat_r)

            featT = work.tile([P, NW], F32, tag="featT")
            nc.vector.transpose(featT[:], ftile[:])

            lg = ps.tile([P, NW], F32, tag="lg")
            nc.tensor.matmul(lg[:], _mm_dt(bda_t[:]), _mm_dt(featT[:]))

            et = work.tile([P, NW], F32, tag="et")
            nc.scalar.activation(et[:], lg[:], EXP, bias=crep_t[:])

            dsum = ps.tile([NPB, NW], F32, tag="dsum")
            nc.tensor.matmul(dsum[:], _mm_dt(bd1_t[:]), _mm_dt(et[:]))
            rcp = work.tile([NPB, NW], F32, tag="rcp")
            nc.vector.reciprocal(rcp[:], dsum[:])

            rcp_b = ps.tile([P, NW], F32, tag="rcpb")
            nc.tensor.matmul(rcp_b[:], _mm_dt(sel_t[:]), _mm_dt(rcp[:]))

            gatesT = work.tile([P, NW], F32, tag="gatesT")
            nc.vector.tensor_mul(gatesT[:], et[:], rcp_b[:])
            wgfT = work.tile([P, NW], F32, tag="wgfT")
            nc.vector.tensor_mul(wgfT[:], featT[:], gatesT[:])

            otile = work.tile([P, RPP * H], F32, tag="oout")
            otile_blk = otile[:].rearrange("p (rb z) -> p rb z", rb=RPP)
            for hb in range(2):
                outT = ps.tile([P, NW], F32, tag=f"outT{hb}")
                nc.tensor.matmul(
                    outT[:], _mm_dt(bdw_t[hb][:]), _mm_dt(wgfT[:]),
                    start=True, stop=False,
                )
                nc.tensor.matmul(
                    outT[:], _mm_dt(bdb_t[hb][:]), _mm_dt(gatesT[:]),
                    start=False, stop=True,
                )
                nc.vector.transpose(
                    otile_blk[:, :, hb * F : (hb + 1) * F], outT[:]
                )
            nc.sync.dma_start(out_r, otile[:])

    nc.compile()
    return nc


def _get_nc():
    if "nc" not in _NC_CACHE:
        _NC_CACHE["nc"] = _build_nc()
    return _NC_CACHE["nc"]


def _prep_params(W_feat, b_feat, W_gate, b_gate):
    wf = np.asarray(W_feat, np.float64)
    wg = np.asarray(W_gate, np.float64).reshape(F, H, F)
    a = np.einsum("fh,fhg->fg", wf, wg)
    c = (
        np.asarray(b_feat, np.float64).reshape(-1) @ np.asarray(W_gate, np.float64)
        + np.asarray(b_gate, np.float64)
    )
    bda = np.zeros((P, P), np.float32)
    for pb in range(NPB):
        bda[pb * F : (pb + 1) * F, pb * F : (pb + 1) * F] = a.astype(np.float32)
    crep = np.tile(c.astype(np.float32).reshape(F, 1), (NPB, 1))
    bd1 = np.zeros((P, NPB), np.float32)
    for pb in range(NPB):
        bd1[pb * F : (pb + 1) * F, pb] = 1.0
    sel = np.zeros((NPB, P), np.float32)
    for pb in range(NPB):
        sel[pb, pb * F : (pb + 1) * F] = 1.0
    wf32 = np.asarray(W_feat, np.float32)
    bf32 = np.asarray(b_feat, np.float32)
    out = {"bdA": bda, "crep": crep, "bd1": bd1, "sel": sel}
    for hb in range(2):
        bdw = np.zeros((P, P), np.float32)
        bdb = np.zeros((P, P), np.float32)
        for pb in range(NPB):
            sl = slice(pb * F, (pb + 1) * F)
            bdw[sl, sl] = wf32[:, hb * F : (hb + 1) * F]
            bdb[sl, sl] = bf32[:, hb * F : (hb + 1) * F]
        out[f"bdw{hb}"] = bdw
        out[f"bdb{hb}"] = bdb
    return out


def kernel(features, W_feat, b_feat, W_gate, b_gate):
    params = _prep_params(W_feat, b_feat, W_gate, b_gate)
    featf = np.ascontiguousarray(np.asarray(features, np.float32).reshape(N, F))
    nc = _get_nc()
    in_maps = [
        {"feat": featf[i * NC_ROWS : (i + 1) * NC_ROWS], **params}
        for i in range(NCORES)
    ]
    res = run_bass_kernel_spmd(nc, in_maps, list(range(NCORES))).results
    out = np.concatenate([res[i]["out"] for i in range(NCORES)], axis=0)
    return out.reshape(B, S, H)


# revision 15
# speedup vs baseline: 2.5666x; 1.5653x over previous
"""VariableSelectionNetwork Trainium2 kernel (8-core data parallel).

Reference computation per row n (F=32 features, H=64 hidden):
    t[n,f,h] = feat[n,f]*W_feat[f,h] + b_feat[f,h]
    gates    = softmax(t.flat @ W_gate + b_gate)        # over f
    out[n,h] = sum_f t[n,f,h] * gates[n,f]

Algebraic collapse (exact, just reassociated):
    logits = feat @ A + c        A[f,g] = sum_h W_feat[f,h]*W_gate[f*H+h,g]
                                 c      = b_feat.flat @ W_gate + b_gate
    e      = exp(logits)         (logits are O(1); no max-shift needed)
    gates  = e * (1 / sum_f e)
    out    = (feat*gates) @ W_feat + gates @ b_feat

Device dataflow per core (2048 rows), "blocked transpose" scheme, 2 slabs:
    ftile[p, (rb f)] <- DMA          (row = p*16+rb; contiguous per partition)
    featT = StreamTranspose(ftile)   -> featT[(pb f), (rb pl)], row=(pb*32+pl)*16+rb
    lg    = blockdiag(A,x4).T @ featT            (one matmul per slab)
    et    = exp(lg + c_rep)                      (ACT, bias per partition)
    dsum  = blockdiag(ones32,x4).T @ et          -> [4, n] row sums over f
    rcp   = reciprocal_approx_fast(dsum)         (DVE custom op)
    rcp_b = gpsimd partition_broadcast per pb    -> [128, n]
    gatesT= et * rcp_b ; wgfT = featT * gatesT   (DVE)
    outT_hb = blockdiag(W_feat[:,hb],x4).T @ wgfT + blockdiag(b_feat[:,hb],x4).T @ gatesT
    otile[p, (rb h)] = StreamTranspose(outT)     -> DMA out
"""

import sys

sys.path.insert(0, "/opt/trn_rl_repo")

import numpy as np

from concourse import bacc, mybir, tile
from concourse.bass_utils import run_bass_kernel_spmd

B, S, F, H = 32, 512, 32, 64
N = B * S
NCORES = 8
NC_ROWS = N // NCORES  # 2048 rows per core
P = 128
NPB = P // F           # 4 partition blocks
RPP = NC_ROWS // P     # 16 rows per partition
NS = 2                 # slabs (pipeline stages over rb)
RBS = RPP // NS        # rb per slab
NW = RBS * F           # transposed-domain columns per slab
F32 = mybir.dt.float32
EXP = mybir.ActivationFunctionType.Exp

# packed param columns
PW = 6 * P + 1         # bdA, bdw0, bdb0, bdw1, bdb1, bd1x, crep
_C_BDA = 0
_C_BDW = [P, 3 * P]
_C_BDB = [2 * P, 4 * P]
_C_BD1 = 5 * P
_C_CREP = 6 * P

_NC_CACHE = {}


def _build_nc():
    nc = bacc.Bacc("TRN2", target_bir_lowering=False, debug=False, num_devices=NCORES)

    feat_d = nc.dram_tensor("feat", [NC_ROWS, F], F32, kind="ExternalInput").ap()
    pp_d = nc.dram_tensor("pp", [P, PW], F32, kind="ExternalInput").ap()
    out_d = nc.dram_tensor("out", [NC_ROWS, H], F32, kind="ExternalOutput").ap()

    with tile.TileContext(nc) as tc:
        with (
            tc.tile_pool(name="const", bufs=1) as cpool,
            tc.tile_pool(name="work", bufs=2) as work,
            tc.tile_pool(name="ps", bufs=2, space="PSUM") as ps,
        ):
            pp = cpool.tile([P, PW], F32)
            nc.scalar.dma_start(pp[:], pp_d)
            bda = pp[:, _C_BDA : _C_BDA + P]
            bdw = [pp[:, c : c + P] for c in _C_BDW]
            bdb = [pp[:, c : c + P] for c in _C_BDB]
            bd1x = pp[:, _C_BD1 : _C_BD1 + P]
            crep = pp[:, _C_CREP : _C_CREP + 1]

            feat_r = feat_d.rearrange("(p r) f -> p (r f)", p=P)   # [128, 512]
            out_r = out_d.rearrange("(p r) h -> p (r h)", p=P)     # [128, 1024]

            for s in range(NS):
                ftile = work.tile([P, NW], F32, tag="fin")
                nc.sync.dma_start(ftile[:], feat_r[:, s * NW : (s + 1) * NW])

                featT = work.tile([P, NW], F32, tag="featT")
                nc.vector.transpose(featT[:], ftile[:])

                lg = ps.tile([P, NW], F32, tag="lg")
                nc.tensor.matmul(lg[:], bda, featT[:])

                et = work.tile([P, NW], F32, tag="et")
                nc.scalar.activation(et[:], lg[:], EXP, bias=crep)

                dsum_b = ps.tile([P, NW], F32, tag="dsum")
                nc.tensor.matmul(dsum_b[:], bd1x, et[:])
                rcp_b = work.tile([P, NW], F32, tag="rcpb")
                nc.vector.reciprocal_approx_fast(rcp_b[:], dsum_b[:])

                gatesT = work.tile([P, NW], F32, tag="gatesT")
                nc.vector.tensor_mul(gatesT[:], et[:], rcp_b[:])
                wgfT = work.tile([P, NW], F32, tag="wgfT")
                nc.vector.tensor_mul(wgfT[:], featT[:], gatesT[:])

                otile = work.tile([P, RBS * H], F32, tag="oout")
                otile_blk = otile[:].rearrange("p (rb z) -> p rb z", rb=RBS)
                for hb in range(2):
                    outT = ps.tile([P, NW], F32, tag=f"outT{hb}")
                    nc.tensor.matmul(outT[:], bdw[hb], wgfT[:], start=True, stop=False)
                    nc.tensor.matmul(outT[:], bdb[hb], gatesT[:], start=False, stop=True)
                    nc.vector.transpose(
                        otile_blk[:, :, hb * F : (hb + 1) * F], outT[:]
                    )
                nc.scalar.dma_start(
                    out_r[:, s * RBS * H : (s + 1) * RBS * H], otile[:]
                )

    nc.compile()
    return nc


def _get_nc():
    if "nc" not in _NC_CACHE:
        _NC_CACHE["nc"] = _build_nc()
    return _NC_CACHE["nc"]


def _prep_params(W_feat, b_feat, W_gate, b_gate):
    wf = np.asarray(W_feat, np.float64)
    wg = np.asarray(W_gate, np.float64).reshape(F, H, F)
    a = np.einsum("fh,fhg->fg", wf, wg).astype(np.float32)
    c = (
        np.asarray(b_feat, np.float64).reshape(-1) @ np.asarray(W_gate, np.float64)
        + np.asarray(b_gate, np.float64)
    ).astype(np.float32)
    wf32 = np.asarray(W_feat, np.float32)
    bf32 = np.asarray(b_feat, np.float32)

    pp = np.zeros((P, PW), np.float32)
    for pb in range(NPB):
        sl = slice(pb * F, (pb + 1) * F)
        pp[sl, _C_BDA + pb * F : _C_BDA + (pb + 1) * F] = a
        for hb in range(2):
            pp[sl, _C_BDW[hb] + pb * F : _C_BDW[hb] + (pb + 1) * F] = (
                wf32[:, hb * F : (hb + 1) * F]
            )
            pp[sl, _C_BDB[hb] + pb * F : _C_BDB[hb] + (pb + 1) * F] = (
                bf32[:, hb * F : (hb + 1) * F]
            )
        pp[sl, _C_BD1 + pb * F : _C_BD1 + (pb + 1) * F] = 1.0
        pp[sl, _C_CREP] = c
    return {"pp": pp}


def kernel(features, W_feat, b_feat, W_gate, b_gate):
    params = _prep_params(W_feat, b_feat, W_gate, b_gate)
    featf = np.ascontiguousarray(np.asarray(features, np.float32).reshape(N, F))
    nc = _get_nc()
    in_maps = [
        {"feat": featf[i * NC_ROWS : (i + 1) * NC_ROWS], **params}
        for i in range(NCORES)
    ]
    res = run_bass_kernel_spmd(nc, in_maps, list(range(NCORES))).results
    out = np.concatenate([res[i]["out"] for i in range(NCORES)], axis=0)
    return out.reshape(B, S, H)


# revision 18
# speedup vs baseline: 2.8408x; 1.1068x over previous
"""VariableSelectionNetwork Trainium2 kernel (8-core data parallel).

Reference computation per row n (F=32 features, H=64 hidden):
    t[n,f,h] = feat[n,f]*W_feat[f,h] + b_feat[f,h]
    gates    = softmax(t.flat @ W_gate + b_gate)        # over f
    out[n,h] = sum_f t[n,f,h] * gates[n,f]

Algebraic collapse (exact, just reassociated):
    logits = feat @ A + c        A[f,g] = sum_h W_feat[f,h]*W_gate[f*H+h,g]
                                 c      = b_feat.flat @ W_gate + b_gate
    e      = exp(logits)         (logits are O(1); no max-shift needed)
    gates  = e * (1 / sum_f e)
    out    = (feat*gates) @ W_feat + gates @ b_feat

Device dataflow per core (2048 rows), "blocked transpose" scheme, 2 slabs:
    ftile[p, (rb f)] <- DMA          (row = p*16+rb; contiguous per partition)
    featT = StreamTranspose(ftile)   -> featT[(pb f), (rb pl)], row=(pb*32+pl)*16+rb
    lg    = blockdiag(A,x4).T @ featT            (one matmul per slab)
    et    = exp(lg + c_rep)                      (ACT, bias per partition)
    dsum  = blockdiag(ones32,x4).T @ et          -> [4, n] row sums over f
    rcp   = reciprocal_approx_fast(dsum)         (DVE custom op)
    rcp_b = gpsimd partition_broadcast per pb    -> [128, n]
    gatesT= et * rcp_b ; wgfT = featT * gatesT   (DVE)
    outT_hb = blockdiag(W_feat[:,hb],x4).T @ wgfT + blockdiag(b_feat[:,hb],x4).T @ gatesT
    otile[p, (rb h)] = StreamTranspose(outT)     -> DMA out
"""

import sys

sys.path.insert(0, "/opt/trn_rl_repo")

import numpy as np

from concourse import bacc, mybir, tile
from concourse.bass_utils import run_bass_kernel_spmd

B, S, F, H = 32, 512, 32, 64
N = B * S
NCORES = 8
NC_ROWS = N // NCORES  # 2048 rows per core
P = 128
NPB = P // F           # 4 partition blocks
RPP = NC_ROWS // P     # 16 rows per partition
NS = 2                 # slabs (pipeline stages over rb)
RBS = RPP // NS        # rb per slab
NW = RBS * F           # transposed-domain columns per slab
F32 = mybir.dt.float32
F32R = mybir.dt.float32r
EXP = mybir.ActivationFunctionType.Exp
MMDT = F32R            # dtype for matmul operands (f32r: 1-pass PE at N>=256)

# packed param columns
PW = 6 * P + 1         # bdA, bdw0, bdb0, bdw1, bdb1, bd1x, crep
_C_BDA = 0
_C_BDW = [P, 3 * P]
_C_BDB = [2 * P, 4 * P]
_C_BD1 = 5 * P
_C_CREP = 6 * P

_NC_CACHE = {}


def _build_nc():
    nc = bacc.Bacc("TRN2", target_bir_lowering=False, debug=False, num_devices=NCORES)

    feat_d = nc.dram_tensor("feat", [NC_ROWS, F], F32, kind="ExternalInput").ap()
    pp_d = nc.dram_tensor("pp", [P, PW], MMDT, kind="ExternalInput").ap()
    out_d = nc.dram_tensor("out", [NC_ROWS, H], F32, kind="ExternalOutput").ap()

    with tile.TileContext(nc) as tc:
        with (
            tc.tile_pool(name="const", bufs=1) as cpool,
            tc.tile_pool(name="work", bufs=2) as work,
            tc.tile_pool(name="ps", bufs=2, space="PSUM") as ps,
        ):
            pp = cpool.tile([P, PW], MMDT)
            nc.scalar.dma_start(pp[:], pp_d)
            bda = pp[:, _C_BDA : _C_BDA + P]
            bdw = [pp[:, c : c + P] for c in _C_BDW]
            bdb = [pp[:, c : c + P] for c in _C_BDB]
            bd1x = pp[:, _C_BD1 : _C_BD1 + P]
            crep = pp[:, _C_CREP : _C_CREP + 1]

            feat_r = feat_d.rearrange("(p r) f -> p (r f)", p=P)   # [128, 512]
            out_r = out_d.rearrange("(p r) h -> p (r h)", p=P)     # [128, 1024]

            for s in range(NS):
                ftile = work.tile([P, NW], F32, tag="fin")
                nc.sync.dma_start(ftile[:], feat_r[:, s * NW : (s + 1) * NW])

                featT = work.tile([P, NW], F32, tag="featT")
                nc.vector.transpose(featT[:], ftile[:])
                featTr = work.tile([P, NW], MMDT, tag="featTr")
                nc.scalar.copy(featTr[:], featT[:])

                lg = ps.tile([P, NW], F32, tag="lg")
                nc.tensor.matmul(lg[:], bda, featTr[:])

                et = work.tile([P, NW], MMDT, tag="et")
                nc.scalar.activation(et[:], lg[:], EXP, bias=crep)

                dsum_b = ps.tile([P, NW], F32, tag="dsum")
                nc.tensor.matmul(dsum_b[:], bd1x, et[:])
                rcp_b = work.tile([P, NW], F32, tag="rcpb")
                nc.vector.reciprocal_approx_fast(rcp_b[:], dsum_b[:])

                gatesT = work.tile([P, NW], MMDT, tag="gatesT")
                nc.vector.tensor_mul(gatesT[:], et[:], rcp_b[:])
                wgfT = work.tile([P, NW], MMDT, tag="wgfT")
                nc.vector.tensor_mul(wgfT[:], featT[:], gatesT[:])

                otile = work.tile([P, RBS * H], F32, tag="oout")
                otile_blk = otile[:].rearrange("p (rb z) -> p rb z", rb=RBS)
                for hb in range(2):
                    outT = ps.tile([P, NW], F32, tag=f"outT{hb}")
                    nc.tensor.matmul(outT[:], bdw[hb], wgfT[:], start=True, stop=False)
                    nc.tensor.matmul(outT[:], bdb[hb], gatesT[:], start=False, stop=True)
                    nc.vector.transpose(
                        otile_blk[:, :, hb * F : (hb + 1) * F], outT[:]
                    )
                nc.scalar.dma_start(
                    out_r[:, s * RBS * H : (s + 1) * RBS * H], otile[:]
                )

    nc.compile()
    return nc


def _get_nc():
    if "nc" not in _NC_CACHE:
        _NC_CACHE["nc"] = _build_nc()
    return _NC_CACHE["nc"]


def _prep_params(W_feat, b_feat, W_gate, b_gate):
    wf = np.asarray(W_feat, np.float64)
    wg = np.asarray(W_gate, np.float64).reshape(F, H, F)
    a = np.einsum("fh,fhg->fg", wf, wg).astype(np.float32)
    c = (
        np.asarray(b_feat, np.float64).reshape(-1) @ np.asarray(W_gate, np.float64)
        + np.asarray(b_gate, np.float64)
    ).astype(np.float32)
    wf32 = np.asarray(W_feat, np.float32)
    bf32 = np.asarray(b_feat, np.float32)

    pp = np.zeros((P, PW), np.float32)
    for pb in range(NPB):
        sl = slice(pb * F, (pb + 1) * F)
        pp[sl, _C_BDA + pb * F : _C_BDA + (pb + 1) * F] = a
        for hb in range(2):
            pp[sl, _C_BDW[hb] + pb * F : _C_BDW[hb] + (pb + 1) * F] = (
                wf32[:, hb * F : (hb + 1) * F]
            )
            pp[sl, _C_BDB[hb] + pb * F : _C_BDB[hb] + (pb + 1) * F] = (
                bf32[:, hb * F : (hb + 1) * F]
            )
        pp[sl, _C_BD1 + pb * F : _C_BD1 + (pb + 1) * F] = 1.0
        pp[sl, _C_CREP] = c
    return {"pp": pp}


def kernel(features, W_feat, b_feat, W_gate, b_gate):
    params = _prep_params(W_feat, b_feat, W_gate, b_gate)
    featf = np.ascontiguousarray(np.asarray(features, np.float32).reshape(N, F))
    nc = _get_nc()
    in_maps = [
        {"feat": featf[i * NC_ROWS : (i + 1) * NC_ROWS], **params}
        for i in range(NCORES)
    ]
    res = run_bass_kernel_spmd(nc, in_maps, list(range(NCORES))).results
    out = np.concatenate([res[i]["out"] for i in range(NCORES)], axis=0)
    return out.reshape(B, S, H)
